# revision 9
# baseline (speedup 1.0000x reference)
"""Depthwise 4x4 FIR blur (upfirdn2d pad=(2,1)) on 8 Trainium2 NeuronCores.

Strategy
--------
Data parallel: shard batch N=32 -> 4 per core. Each core processes 1024
independent 64x64 images (4 batches x 256 channels).

Per-core compute: the 2D conv is decomposed (via SVD of the 4x4 tap
kernel) into a sum of separable rank-1 terms (the real blur kernel is
exactly rank 1). Each rank term is two banded-Toeplitz matmuls on the
TensorEngine:

  pass 1 (H-conv):  psum1 = X.T @ TC2      (contract over image rows)
  pass 2 (W-conv):  psum2 += Q.T @ TR2     (contract over image cols)

where X packs 4 images in quadrants of a [128,128] tile
(partition = kb*64 + row, free = mb*64 + col), TC2/TR2 =
blockdiag(Toeplitz64, Toeplitz64) constants, and Q is the PSUM->SBUF
copy of psum1. Both passes keep data in natural layout (the data tile
is the stationary operand), so no transposes are needed and psum2 holds
the output images in the same quadrant layout as the input. Zero padding
is implicit in the band truncation of the [64,64] Toeplitz matrices.

I/O: the host pre-permutes x into a [128, 32768] per-core layout in
which every SBUF slab is contiguous per partition, so each DMA moves
a large slab across all 128 partitions at full port bandwidth. The
inverse permutation is applied to the output on the host.

Precision: the whole device path runs in bf16 (I/O tensors, Toeplitz
constants, intermediate Q) with fp32 PSUM accumulation. The blur taps
([1,3,3,1]-separable => SVD factors [0.25,0.75,0.75,0.25]) are exactly
representable in bf16, so the only error is input/intermediate/output
rounding: ~0.5% max-rel, well inside the 2e-2 gate. This halves HBM
traffic (the DMA floor) and runs the PE at 4x the fp32 matmul rate.

Pipelining: pass 2 runs one supergroup behind pass 1 (software skew) so
the TensorEngine never stalls on the PSUM->SBUF copy between passes.
"""

import functools
import math

import ml_dtypes
import numpy as np

import concourse.bacc as bacc
import concourse.tile as tile
from concourse import mybir
from concourse.bass_utils import run_bass_kernel_spmd

N_CORES = 8
N, C, H, W = 32, 256, 64, 64
PER_CORE = N // N_CORES        # 4 batch entries per core
IMGS = PER_CORE * C            # 1024 images per core
SG = 16                        # images per supergroup (4 quads)
NSG = IMGS // SG               # 64 supergroups per core
SPB = 4                        # supergroups per DMA slab (1 MiB)
NSLAB = NSG // SPB


def _toeplitz64(vec4):
    """T[p, i] = vec4[1 + i - p] when 0 <= 1+i-p <= 3 else 0 ([64, 64])."""
    T = np.zeros((64, 64), np.float64)
    for a in range(4):
        k = a - 1
        T += np.diag(np.full(64 - abs(k), vec4[a]), k=k)
    return T


def _permute_in(x_core):
    """[1024, 64, 64] -> [128, NSG*512] host layout.

    Supergroup s holds images 16s..16s+15; image (kb, q) = 16s + 8kb + q
    lands at SBUF partition 64kb+row, free s*512 + q*64 + col."""
    v = x_core.reshape(NSG, 2, 8, 64, 64)           # [s, kb, q, p, w]
    v = v.transpose(1, 3, 0, 2, 4)                  # [kb, p, s, q, w]
    return np.ascontiguousarray(
        v.reshape(128, NSG * 512).astype(ml_dtypes.bfloat16))


def _permute_out(o_perm):
    """Inverse of _permute_in for the output buffer (bf16 -> fp32)."""
    v = o_perm.astype(np.float32).reshape(2, 64, NSG, 8, 64)  # [kb,i,s,q,j]
    v = v.transpose(2, 0, 3, 1, 4)                  # [s, kb, q, i, j]
    return v.reshape(IMGS, 64, 64)


@functools.lru_cache(maxsize=8)
def _build(rank, loops=1, dyn_loop=False):
    """Build + compile the per-core bass program (same NEFF on all cores).

    dyn_loop=True (benchmark-only) wraps the computation in a hardware
    For_i loop whose trip count comes from an extra `nrep` input, so one
    executable can measure any repetition count (wall-time slope vs nrep
    isolates per-execution HW time from dispatch overhead)."""
    import concourse.bass as bass
    nc = bacc.Bacc("TRN2", target_bir_lowering=False, debug=False)
    dt = mybir.dt.bfloat16
    dt32 = mybir.dt.float32
    xp = nc.dram_tensor("xp", [128, NSG * 512], dt, kind="ExternalInput").ap()
    tcol = nc.dram_tensor("tcol", [rank, 128, 128], dt, kind="ExternalInput").ap()
    trow = nc.dram_tensor("trow", [rank, 128, 128], dt, kind="ExternalInput").ap()
    op = nc.dram_tensor("op", [128, NSG * 512], dt, kind="ExternalOutput").ap()
    if dyn_loop:
        nrep = nc.dram_tensor("nrep", [1, 1], mybir.dt.int32,
                              kind="ExternalInput").ap()

    with tile.TileContext(nc) as tc:
        with (
            tc.tile_pool(name="consts", bufs=1) as cpool,
            tc.tile_pool(name="xin", bufs=3) as xpool,
            tc.tile_pool(name="q", bufs=6) as qpool,
            tc.tile_pool(name="o", bufs=3) as opool,
            tc.tile_pool(name="ps1", bufs=3, space="PSUM") as ps1pool,
            tc.tile_pool(name="ps2", bufs=3, space="PSUM") as ps2pool,
        ):
            tcs, trs = [], []
            for r in range(rank):
                tct = cpool.tile([128, 128], dt, tag=f"tc{r}")
                nc.sync.dma_start(tct[:], tcol[r])
                trt = cpool.tile([128, 128], dt, tag=f"tr{r}")
                nc.sync.dma_start(trt[:], trow[r])
                tcs.append(tct)
                trs.append(trt)

            import contextlib
            loop_cm = contextlib.nullcontext()
            if dyn_loop:
                cnt = cpool.tile([1, 1], mybir.dt.int32, tag="cnt")
                cnt_sem = nc.alloc_semaphore("cnt_sem")
                with tc.tile_critical():
                    nc.sync.dma_start(cnt[:], nrep[:]).then_inc(cnt_sem, 16)
                    regs = []
                    for e in mybir.ALL_ENGINES:
                        rr = nc.alloc_register(e, f"cnt_{e.name}")
                        nc.engines[e].reg_load(rr, cnt[0:1, 0:1])._wait_ge(
                            cnt_sem, 16)
                        regs.append(rr)
                rv = nc.snap(bass.RegisterHandles(regs))
                loop_cm = tc.For_i(0, rv, 1)

            slabs = {}     # slab idx -> [X, O, n_supergroups_done]
            pending = []   # (slab idx, sl, [Q_r ...]) awaiting pass 2

            CS = 320   # copy split point: cols [0:CS] on DVE, [CS:512] on ACT

            def do_pass1(k, sl):
                X = slabs[k][0]
                qs = []
                for r in range(rank):
                    ps1 = ps1pool.tile([128, 512], dt32, tag="ps1")
                    for t in range(4):
                        nc.tensor.matmul(
                            ps1[:, 128 * t:128 * (t + 1)],
                            X[:, sl * 512 + 128 * t: sl * 512 + 128 * (t + 1)],
                            tcs[r][:], start=True, stop=True)
                    Q = qpool.tile([128, 512], dt, tag="Q")
                    nc.vector.tensor_copy(Q[:, :CS], ps1[:, :CS])
                    nc.scalar.copy(Q[:, CS:], ps1[:, CS:])
                    qs.append(Q)
                pending.append((k, sl, qs))

            def do_pass2():
                k, sl, qs = pending.pop(0)
                ps2 = ps2pool.tile([128, 512], dt32, tag="ps2")
                # complete each column-region's accumulation group over all
                # ranks before opening the next (PSUM allows one pending
                # group per zero region)
                for t in range(4):
                    for r in range(rank):
                        nc.tensor.matmul(
                            ps2[:, 128 * t:128 * (t + 1)],
                            qs[r][:, 128 * t:128 * (t + 1)],
                            trs[r][:], start=(r == 0), stop=(r == rank - 1))
                O = slabs[k][1]
                dst = O[:, sl * 512:(sl + 1) * 512]
                nc.vector.tensor_copy(dst[:, :CS], ps2[:, :CS])
                nc.scalar.copy(dst[:, CS:], ps2[:, CS:])
                slabs[k][2] += 1
                if slabs[k][2] == SPB:
                    kk = k % NSLAB
                    nc.scalar.dma_start(
                        op[:, kk * SPB * 512:(kk + 1) * SPB * 512], O[:])
                    del slabs[k]

            with loop_cm:
                for k in range(NSLAB * loops):
                    X = xpool.tile([128, SPB * 512], dt, tag="X")
                    kk = k % NSLAB
                    nc.sync.dma_start(
                        X[:], xp[:, kk * SPB * 512:(kk + 1) * SPB * 512])
                    O = opool.tile([128, SPB * 512], dt, tag="O")
                    slabs[k] = [X, O, 0]
                    for sl in range(SPB):
                        do_pass1(k, sl)
                        if len(pending) > 2:
                            do_pass2()
                while pending:
                    do_pass2()
    nc.compile()
    return nc


def _decompose(k):
    """SVD rank decomposition of the 4x4 tap kernel into blockdiag
    Toeplitz constant pairs (tcol[r], trow[r]) of shape [128, 128]."""
    U, S, Vt = np.linalg.svd(np.asarray(k, np.float64))
    rank = max(1, int((S > S[0] * 1e-9).sum())) if S[0] > 0 else 1
    tcs = np.zeros((rank, 128, 128), ml_dtypes.bfloat16)
    trs = np.zeros((rank, 128, 128), ml_dtypes.bfloat16)
    for r in range(rank):
        u = U[:, r] * math.sqrt(S[r])
        v = Vt[r, :] * math.sqrt(S[r])
        Tc = _toeplitz64(u).astype(np.float32)
        Tr = _toeplitz64(v).astype(np.float32)
        tcs[r, :64, :64] = Tc
        tcs[r, 64:, 64:] = Tc
        trs[r, :64, :64] = Tr
        trs[r, 64:, 64:] = Tr
    return tcs, trs


def run(x, k, trace=False, loops=1):
    """Run the blur on 8 cores. Returns (out, BassKernelResults)."""
    x = np.asarray(x, dtype=np.float32)
    k = np.asarray(k, dtype=np.float32)
    assert x.shape == (N, C, H, W), x.shape
    assert k.shape == (4, 4), k.shape
    tcs, trs = _decompose(k)
    nc = _build(tcs.shape[0], loops)
    in_maps = [
        {
            "xp": _permute_in(x[i * PER_CORE:(i + 1) * PER_CORE].reshape(IMGS, H, W)),
            "tcol": tcs,
            "trow": trs,
        }
        for i in range(N_CORES)
    ]
    res = run_bass_kernel_spmd(nc, in_maps, core_ids=list(range(N_CORES)),
                               trace=trace)
    out = np.concatenate(
        [
            _permute_out(r["op"]).reshape(PER_CORE, C, H, W)
            for r in res.results
        ],
        axis=0,
    )
    return out, res


def kernel(x, kernel):
    return run(x, kernel)[0]



# revision 11
# speedup vs baseline: 1.0888x; 1.0888x over previous
"""Depthwise 4x4 FIR blur (upfirdn2d pad=(2,1)) on 8 Trainium2 NeuronCores.

Strategy
--------
Data parallel: shard batch N=32 -> 4 per core. Each core processes 1024
independent 64x64 images (4 batches x 256 channels).

Per-core compute: the 2D conv is decomposed (via SVD of the 4x4 tap
kernel) into a sum of separable rank-1 terms (the real blur kernel is
exactly rank 1). Each rank term is two banded-Toeplitz matmuls on the
TensorEngine:

  pass 1 (H-conv):  psum1 = X.T @ TC2      (contract over image rows)
  pass 2 (W-conv):  psum2 += Q.T @ TR2     (contract over image cols)

where X packs 4 images in quadrants of a [128,128] tile
(partition = kb*64 + row, free = mb*64 + col), TC2/TR2 =
blockdiag(Toeplitz64, Toeplitz64) constants, and Q is the PSUM->SBUF
copy of psum1. Both passes keep data in natural layout (the data tile
is the stationary operand), so no transposes are needed and psum2 holds
the output images in the same quadrant layout as the input. Zero padding
is implicit in the band truncation of the [64,64] Toeplitz matrices.

I/O: the host pre-permutes x into a [128, 32768] per-core layout in
which every SBUF slab is contiguous per partition, so each DMA moves
a large slab across all 128 partitions at full port bandwidth. The
inverse permutation is applied to the output on the host.

Precision: the whole device path runs in bf16 (I/O tensors, Toeplitz
constants, intermediate Q) with fp32 PSUM accumulation. The blur taps
([1,3,3,1]-separable => SVD factors [0.25,0.75,0.75,0.25]) are exactly
representable in bf16, so the only error is input/intermediate/output
rounding: ~0.5% max-rel, well inside the 2e-2 gate. This halves HBM
traffic (the DMA floor) and runs the PE at 4x the fp32 matmul rate.

Pipelining: pass 2 runs one supergroup behind pass 1 (software skew) so
the TensorEngine never stalls on the PSUM->SBUF copy between passes.
"""

import functools
import math

import ml_dtypes
import numpy as np

import concourse.bacc as bacc
import concourse.tile as tile
from concourse import mybir
from concourse.bass_utils import run_bass_kernel_spmd

N_CORES = 8
N, C, H, W = 32, 256, 64, 64
PER_CORE = N // N_CORES        # 4 batch entries per core
IMGS = PER_CORE * C            # 1024 images per core
SG = 16                        # images per supergroup (4 quads)
NSG = IMGS // SG               # 64 supergroups per core
SPB = 4                        # supergroups per DMA slab (1 MiB)
NSLAB = NSG // SPB


def _toeplitz64(vec4):
    """T[p, i] = vec4[1 + i - p] when 0 <= 1+i-p <= 3 else 0 ([64, 64])."""
    T = np.zeros((64, 64), np.float64)
    for a in range(4):
        k = a - 1
        T += np.diag(np.full(64 - abs(k), vec4[a]), k=k)
    return T


def _permute_in(x_core):
    """[1024, 64, 64] -> [128, NSG*512] host layout.

    Supergroup s holds images 16s..16s+15; image (kb, q) = 16s + 8kb + q
    lands at SBUF partition 64kb+row, free s*512 + q*64 + col."""
    v = x_core.reshape(NSG, 2, 8, 64, 64)           # [s, kb, q, p, w]
    v = v.transpose(1, 3, 0, 2, 4)                  # [kb, p, s, q, w]
    return np.ascontiguousarray(
        v.reshape(128, NSG * 512).astype(ml_dtypes.bfloat16))


def _permute_out(o_perm):
    """Inverse of _permute_in for the output buffer (bf16 -> fp32)."""
    v = o_perm.astype(np.float32).reshape(2, 64, NSG, 8, 64)  # [kb,i,s,q,j]
    v = v.transpose(2, 0, 3, 1, 4)                  # [s, kb, q, i, j]
    return v.reshape(IMGS, 64, 64)


@functools.lru_cache(maxsize=8)
def _build(rank, loops=1, dyn_loop=False):
    """Build + compile the per-core bass program (same NEFF on all cores).

    dyn_loop=True (benchmark-only) wraps the computation in a hardware
    For_i loop whose trip count comes from an extra `nrep` input, so one
    executable can measure any repetition count (wall-time slope vs nrep
    isolates per-execution HW time from dispatch overhead)."""
    import concourse.bass as bass
    nc = bacc.Bacc("TRN2", target_bir_lowering=False, debug=False)
    dt = mybir.dt.bfloat16
    dt32 = mybir.dt.float32
    xp = nc.dram_tensor("xp", [128, NSG * 512], dt, kind="ExternalInput").ap()
    tcol = nc.dram_tensor("tcol", [rank, 128, 128], dt, kind="ExternalInput").ap()
    trow = nc.dram_tensor("trow", [rank, 128, 128], dt, kind="ExternalInput").ap()
    op = nc.dram_tensor("op", [128, NSG * 512], dt, kind="ExternalOutput").ap()
    if dyn_loop:
        nrep = nc.dram_tensor("nrep", [1, 1], mybir.dt.int32,
                              kind="ExternalInput").ap()

    with tile.TileContext(nc) as tc:
        with (
            tc.tile_pool(name="consts", bufs=1) as cpool,
            tc.tile_pool(name="xin", bufs=3) as xpool,
            tc.tile_pool(name="q", bufs=6) as qpool,
            tc.tile_pool(name="o", bufs=3) as opool,
            tc.tile_pool(name="ps1", bufs=3, space="PSUM") as ps1pool,
            tc.tile_pool(name="ps2", bufs=3, space="PSUM") as ps2pool,
        ):
            tcs, trs = [], []
            for r in range(rank):
                tct = cpool.tile([128, 128], dt, tag=f"tc{r}")
                nc.sync.dma_start(tct[:], tcol[r])
                trt = cpool.tile([128, 128], dt, tag=f"tr{r}")
                nc.sync.dma_start(trt[:], trow[r])
                tcs.append(tct)
                trs.append(trt)

            import contextlib
            loop_cm = contextlib.nullcontext()
            if dyn_loop:
                cnt = cpool.tile([1, 1], mybir.dt.int32, tag="cnt")
                cnt_sem = nc.alloc_semaphore("cnt_sem")
                with tc.tile_critical():
                    nc.sync.dma_start(cnt[:], nrep[:]).then_inc(cnt_sem, 16)
                    regs = []
                    for e in mybir.ALL_ENGINES:
                        rr = nc.alloc_register(e, f"cnt_{e.name}")
                        nc.engines[e].reg_load(rr, cnt[0:1, 0:1])._wait_ge(
                            cnt_sem, 16)
                        regs.append(rr)
                rv = nc.snap(bass.RegisterHandles(regs))
                loop_cm = tc.For_i(0, rv, 1)

            slabs = {}     # slab idx -> [X, O, n_supergroups_done]
            pending = []   # (slab idx, sl, [Q_r ...]) awaiting pass 2

            # PSUM->SBUF copies: each [128,512] fp32 tile goes WHOLE to one
            # engine, greedily balancing modeled busy-time. DVE whole-tile
            # cost (0.96 GHz, 120+FD cyc + pipe-drain dur-266ns) ~1050ns;
            # ACT (1.2 GHz, 172+FD cyc) ~570ns -> ~45:83 tile interleave,
            # ~47us/core aggregate, overlapping the DMA floor.
            eng_t = {"v": 0.0, "s": 0.0}
            DVE_TILE_NS, ACT_TILE_NS = 1050.0, 570.0

            def copy_tile(dst, src):
                if eng_t["v"] + DVE_TILE_NS <= eng_t["s"] + ACT_TILE_NS:
                    eng_t["v"] += DVE_TILE_NS
                    nc.vector.tensor_copy(dst, src)
                else:
                    eng_t["s"] += ACT_TILE_NS
                    nc.scalar.copy(dst, src)

            def do_pass1(k, sl):
                X = slabs[k][0]
                qs = []
                for r in range(rank):
                    ps1 = ps1pool.tile([128, 512], dt32, tag="ps1")
                    for t in range(4):
                        nc.tensor.matmul(
                            ps1[:, 128 * t:128 * (t + 1)],
                            X[:, sl * 512 + 128 * t: sl * 512 + 128 * (t + 1)],
                            tcs[r][:], start=True, stop=True)
                    Q = qpool.tile([128, 512], dt, tag="Q")
                    copy_tile(Q[:], ps1[:])
                    qs.append(Q)
                pending.append((k, sl, qs))

            def do_pass2():
                k, sl, qs = pending.pop(0)
                ps2 = ps2pool.tile([128, 512], dt32, tag="ps2")
                # complete each column-region's accumulation group over all
                # ranks before opening the next (PSUM allows one pending
                # group per zero region)
                for t in range(4):
                    for r in range(rank):
                        nc.tensor.matmul(
                            ps2[:, 128 * t:128 * (t + 1)],
                            qs[r][:, 128 * t:128 * (t + 1)],
                            trs[r][:], start=(r == 0), stop=(r == rank - 1))
                O = slabs[k][1]
                dst = O[:, sl * 512:(sl + 1) * 512]
                copy_tile(dst, ps2[:])
                slabs[k][2] += 1
                if slabs[k][2] == SPB:
                    kk = k % NSLAB
                    nc.scalar.dma_start(
                        op[:, kk * SPB * 512:(kk + 1) * SPB * 512], O[:])
                    del slabs[k]

            with loop_cm:
                for k in range(NSLAB * loops):
                    X = xpool.tile([128, SPB * 512], dt, tag="X")
                    kk = k % NSLAB
                    nc.sync.dma_start(
                        X[:], xp[:, kk * SPB * 512:(kk + 1) * SPB * 512])
                    O = opool.tile([128, SPB * 512], dt, tag="O")
                    slabs[k] = [X, O, 0]
                    for sl in range(SPB):
                        do_pass1(k, sl)
                        if len(pending) > 2:
                            do_pass2()
                while pending:
                    do_pass2()
    nc.compile()
    return nc


def _decompose(k):
    """SVD rank decomposition of the 4x4 tap kernel into blockdiag
    Toeplitz constant pairs (tcol[r], trow[r]) of shape [128, 128]."""
    U, S, Vt = np.linalg.svd(np.asarray(k, np.float64))
    rank = max(1, int((S > S[0] * 1e-9).sum())) if S[0] > 0 else 1
    tcs = np.zeros((rank, 128, 128), ml_dtypes.bfloat16)
    trs = np.zeros((rank, 128, 128), ml_dtypes.bfloat16)
    for r in range(rank):
        u = U[:, r] * math.sqrt(S[r])
        v = Vt[r, :] * math.sqrt(S[r])
        Tc = _toeplitz64(u).astype(np.float32)
        Tr = _toeplitz64(v).astype(np.float32)
        tcs[r, :64, :64] = Tc
        tcs[r, 64:, 64:] = Tc
        trs[r, :64, :64] = Tr
        trs[r, 64:, 64:] = Tr
    return tcs, trs


def run(x, k, trace=False, loops=1):
    """Run the blur on 8 cores. Returns (out, BassKernelResults)."""
    x = np.asarray(x, dtype=np.float32)
    k = np.asarray(k, dtype=np.float32)
    assert x.shape == (N, C, H, W), x.shape
    assert k.shape == (4, 4), k.shape
    tcs, trs = _decompose(k)
    nc = _build(tcs.shape[0], loops)
    in_maps = [
        {
            "xp": _permute_in(x[i * PER_CORE:(i + 1) * PER_CORE].reshape(IMGS, H, W)),
            "tcol": tcs,
            "trow": trs,
        }
        for i in range(N_CORES)
    ]
    res = run_bass_kernel_spmd(nc, in_maps, core_ids=list(range(N_CORES)),
                               trace=trace)
    out = np.concatenate(
        [
            _permute_out(r["op"]).reshape(PER_CORE, C, H, W)
            for r in res.results
        ],
        axis=0,
    )
    return out, res


def kernel(x, kernel):
    return run(x, kernel)[0]



# revision 13
# speedup vs baseline: 1.1918x; 1.0946x over previous
"""Depthwise 4x4 FIR blur (upfirdn2d pad=(2,1)) on 8 Trainium2 NeuronCores.

Strategy
--------
Data parallel: shard batch N=32 -> 4 per core. Each core processes 1024
independent 64x64 images (4 batches x 256 channels).

Per-core compute: the 2D conv is decomposed (via SVD of the 4x4 tap
kernel) into a sum of separable rank-1 terms (the real blur kernel is
exactly rank 1). Each rank term is two banded-Toeplitz matmuls on the
TensorEngine:

  pass 1 (H-conv):  psum1 = X.T @ TC2      (contract over image rows)
  pass 2 (W-conv):  psum2 += Q.T @ TR2     (contract over image cols)

where X packs 4 images in quadrants of a [128,128] tile
(partition = kb*64 + row, free = mb*64 + col), TC2/TR2 =
blockdiag(Toeplitz64, Toeplitz64) constants, and Q is the PSUM->SBUF
copy of psum1. Both passes keep data in natural layout (the data tile
is the stationary operand), so no transposes are needed and psum2 holds
the output images in the same quadrant layout as the input. Zero padding
is implicit in the band truncation of the [64,64] Toeplitz matrices.

I/O: the host pre-permutes x into a [128, 32768] per-core layout in
which every SBUF slab is contiguous per partition, so each DMA moves
a large slab across all 128 partitions at full port bandwidth. The
inverse permutation is applied to the output on the host.

Precision: the whole device path runs in bf16 (I/O tensors, Toeplitz
constants, intermediate Q) with fp32 PSUM accumulation. The blur taps
([1,3,3,1]-separable => SVD factors [0.25,0.75,0.75,0.25]) are exactly
representable in bf16, so the only error is input/intermediate/output
rounding: ~0.5% max-rel, well inside the 2e-2 gate. This halves HBM
traffic (the DMA floor) and runs the PE at 4x the fp32 matmul rate.

Pipelining: pass 2 runs one supergroup behind pass 1 (software skew) so
the TensorEngine never stalls on the PSUM->SBUF copy between passes.
"""

import functools
import math

import ml_dtypes
import numpy as np

import concourse.bacc as bacc
import concourse.tile as tile
from concourse import mybir
from concourse.bass_utils import run_bass_kernel_spmd

N_CORES = 8
N, C, H, W = 32, 256, 64, 64
PER_CORE = N // N_CORES        # 4 batch entries per core
IMGS = PER_CORE * C            # 1024 images per core
SG = 16                        # images per supergroup (4 quads)
NSG = IMGS // SG               # 64 supergroups per core
SPB = 8                        # supergroups per DMA slab (1 MiB bf16)
NSLAB = NSG // SPB


def _toeplitz64(vec4):
    """T[p, i] = vec4[1 + i - p] when 0 <= 1+i-p <= 3 else 0 ([64, 64])."""
    T = np.zeros((64, 64), np.float64)
    for a in range(4):
        k = a - 1
        T += np.diag(np.full(64 - abs(k), vec4[a]), k=k)
    return T


def _permute_in(x_core):
    """[1024, 64, 64] -> [128, NSG*512] host layout.

    Supergroup s holds images 16s..16s+15; image (kb, q) = 16s + 8kb + q
    lands at SBUF partition 64kb+row, free s*512 + q*64 + col."""
    v = x_core.reshape(NSG, 2, 8, 64, 64)           # [s, kb, q, p, w]
    v = v.transpose(1, 3, 0, 2, 4)                  # [kb, p, s, q, w]
    return np.ascontiguousarray(
        v.reshape(128, NSG * 512).astype(ml_dtypes.bfloat16))


def _permute_out(o_perm):
    """Inverse of _permute_in for the output buffer (bf16 -> fp32)."""
    v = o_perm.astype(np.float32).reshape(2, 64, NSG, 8, 64)  # [kb,i,s,q,j]
    v = v.transpose(2, 0, 3, 1, 4)                  # [s, kb, q, i, j]
    return v.reshape(IMGS, 64, 64)


@functools.lru_cache(maxsize=8)
def _build(rank, loops=1, dyn_loop=False):
    """Build + compile the per-core bass program (same NEFF on all cores).

    dyn_loop=True (benchmark-only) wraps the computation in a hardware
    For_i loop whose trip count comes from an extra `nrep` input, so one
    executable can measure any repetition count (wall-time slope vs nrep
    isolates per-execution HW time from dispatch overhead)."""
    import concourse.bass as bass
    nc = bacc.Bacc("TRN2", target_bir_lowering=False, debug=False)
    dt = mybir.dt.bfloat16
    dt32 = mybir.dt.float32
    xp = nc.dram_tensor("xp", [128, NSG * 512], dt, kind="ExternalInput").ap()
    tcol = nc.dram_tensor("tcol", [rank, 128, 128], dt, kind="ExternalInput").ap()
    trow = nc.dram_tensor("trow", [rank, 128, 128], dt, kind="ExternalInput").ap()
    op = nc.dram_tensor("op", [128, NSG * 512], dt, kind="ExternalOutput").ap()
    if dyn_loop:
        nrep = nc.dram_tensor("nrep", [1, 1], mybir.dt.int32,
                              kind="ExternalInput").ap()

    with tile.TileContext(nc) as tc:
        with (
            tc.tile_pool(name="consts", bufs=1) as cpool,
            tc.tile_pool(name="xin", bufs=3) as xpool,
            tc.tile_pool(name="q", bufs=6) as qpool,
            tc.tile_pool(name="o", bufs=3) as opool,
            tc.tile_pool(name="ps1", bufs=3, space="PSUM") as ps1pool,
            tc.tile_pool(name="ps2", bufs=3, space="PSUM") as ps2pool,
        ):
            tcs, trs = [], []
            for r in range(rank):
                tct = cpool.tile([128, 128], dt, tag=f"tc{r}")
                nc.sync.dma_start(tct[:], tcol[r])
                trt = cpool.tile([128, 128], dt, tag=f"tr{r}")
                nc.sync.dma_start(trt[:], trow[r])
                tcs.append(tct)
                trs.append(trt)

            import contextlib
            loop_cm = contextlib.nullcontext()
            if dyn_loop:
                cnt = cpool.tile([1, 1], mybir.dt.int32, tag="cnt")
                cnt_sem = nc.alloc_semaphore("cnt_sem")
                with tc.tile_critical():
                    nc.sync.dma_start(cnt[:], nrep[:]).then_inc(cnt_sem, 16)
                    regs = []
                    for e in mybir.ALL_ENGINES:
                        rr = nc.alloc_register(e, f"cnt_{e.name}")
                        nc.engines[e].reg_load(rr, cnt[0:1, 0:1])._wait_ge(
                            cnt_sem, 16)
                        regs.append(rr)
                rv = nc.snap(bass.RegisterHandles(regs))
                loop_cm = tc.For_i(0, rv, 1)

            slabs = {}     # slab idx -> [X, O, n_supergroups_done]
            pending = []   # (slab idx, sl, [Q_r ...]) awaiting pass 2

            # PSUM->SBUF copies: each [128,512] fp32 tile goes WHOLE to one
            # engine, greedily balancing modeled busy-time. HW-measured
            # (slope microbench): both DVE and ACT sustain ~607ns per
            # whole-tile copy back-to-back -> 50:50 alternation, ~39us/core
            # aggregate, under the DMA floor.
            eng_t = {"v": 0.0, "s": 0.0}
            DVE_TILE_NS, ACT_TILE_NS = 607.0, 607.0

            def copy_tile(dst, src):
                if eng_t["v"] + DVE_TILE_NS <= eng_t["s"] + ACT_TILE_NS:
                    eng_t["v"] += DVE_TILE_NS
                    nc.vector.tensor_copy(dst, src)
                else:
                    eng_t["s"] += ACT_TILE_NS
                    nc.scalar.copy(dst, src)

            def do_pass1(k, sl):
                X = slabs[k][0]
                qs = []
                for r in range(rank):
                    ps1 = ps1pool.tile([128, 512], dt32, tag="ps1")
                    for t in range(4):
                        nc.tensor.matmul(
                            ps1[:, 128 * t:128 * (t + 1)],
                            X[:, sl * 512 + 128 * t: sl * 512 + 128 * (t + 1)],
                            tcs[r][:], start=True, stop=True)
                    Q = qpool.tile([128, 512], dt, tag="Q")
                    copy_tile(Q[:], ps1[:])
                    qs.append(Q)
                pending.append((k, sl, qs))

            def do_pass2():
                k, sl, qs = pending.pop(0)
                ps2 = ps2pool.tile([128, 512], dt32, tag="ps2")
                # complete each column-region's accumulation group over all
                # ranks before opening the next (PSUM allows one pending
                # group per zero region)
                for t in range(4):
                    for r in range(rank):
                        nc.tensor.matmul(
                            ps2[:, 128 * t:128 * (t + 1)],
                            qs[r][:, 128 * t:128 * (t + 1)],
                            trs[r][:], start=(r == 0), stop=(r == rank - 1))
                O = slabs[k][1]
                dst = O[:, sl * 512:(sl + 1) * 512]
                copy_tile(dst, ps2[:])
                slabs[k][2] += 1
                if slabs[k][2] == SPB:
                    kk = k % NSLAB
                    nc.scalar.dma_start(
                        op[:, kk * SPB * 512:(kk + 1) * SPB * 512], O[:])
                    del slabs[k]

            with loop_cm:
                for k in range(NSLAB * loops):
                    X = xpool.tile([128, SPB * 512], dt, tag="X")
                    kk = k % NSLAB
                    nc.sync.dma_start(
                        X[:], xp[:, kk * SPB * 512:(kk + 1) * SPB * 512])
                    O = opool.tile([128, SPB * 512], dt, tag="O")
                    slabs[k] = [X, O, 0]
                    for sl in range(SPB):
                        do_pass1(k, sl)
                        if len(pending) > 2:
                            do_pass2()
                while pending:
                    do_pass2()
    nc.compile()
    return nc


def _decompose(k):
    """SVD rank decomposition of the 4x4 tap kernel into blockdiag
    Toeplitz constant pairs (tcol[r], trow[r]) of shape [128, 128]."""
    U, S, Vt = np.linalg.svd(np.asarray(k, np.float64))
    rank = max(1, int((S > S[0] * 1e-9).sum())) if S[0] > 0 else 1
    tcs = np.zeros((rank, 128, 128), ml_dtypes.bfloat16)
    trs = np.zeros((rank, 128, 128), ml_dtypes.bfloat16)
    for r in range(rank):
        u = U[:, r] * math.sqrt(S[r])
        v = Vt[r, :] * math.sqrt(S[r])
        Tc = _toeplitz64(u).astype(np.float32)
        Tr = _toeplitz64(v).astype(np.float32)
        tcs[r, :64, :64] = Tc
        tcs[r, 64:, 64:] = Tc
        trs[r, :64, :64] = Tr
        trs[r, 64:, 64:] = Tr
    return tcs, trs


def run(x, k, trace=False, loops=1):
    """Run the blur on 8 cores. Returns (out, BassKernelResults)."""
    x = np.asarray(x, dtype=np.float32)
    k = np.asarray(k, dtype=np.float32)
    assert x.shape == (N, C, H, W), x.shape
    assert k.shape == (4, 4), k.shape
    tcs, trs = _decompose(k)
    nc = _build(tcs.shape[0], loops)
    in_maps = [
        {
            "xp": _permute_in(x[i * PER_CORE:(i + 1) * PER_CORE].reshape(IMGS, H, W)),
            "tcol": tcs,
            "trow": trs,
        }
        for i in range(N_CORES)
    ]
    res = run_bass_kernel_spmd(nc, in_maps, core_ids=list(range(N_CORES)),
                               trace=trace)
    out = np.concatenate(
        [
            _permute_out(r["op"]).reshape(PER_CORE, C, H, W)
            for r in res.results
        ],
        axis=0,
    )
    return out, res


def kernel(x, kernel):
    return run(x, kernel)[0]

